# revision 27
# baseline (speedup 1.0000x reference)
"""Trainium2 Bass kernel for nn_Attention_34806414967022 (sparse channel attention).

Data-parallel over batch: 8 batch images -> 8 NeuronCores, one image each.

Per-core pipeline (image = (C=128, H=128, W=128), L = H*W = 16384):
  1. Depthwise-3x3(Conv1x1(x)) planes, fully folded into PE matmuls
     (W_eff[tap] = diag(w_dw[:,tap]) @ W_qkv applied to shifted x views):
       - q,k planes: fp8(e4m3) weights+activations, taps fused in pairs via
         MatmulPerfMode.DoubleRow (2 taps per instruction at 2x rate).
         Weights pre-scaled by 2^13 on host, descaled in the PSUM evacuation.
       - v plane: bf16 (accuracy-critical: attn@v feeds the output directly).
     Shift safety: x chunks live in a gap layout (row stride 130, two zero gap
     cols) so +-1 column shifts read zeros at row edges. x arrives pre-cast
     from the host (bf16 + fp8 copies) and is DMAed straight into the gap
     layout - no on-device cast.
  2. Channel stats: row sums of q,k are computed EXACTLY on the host via
     rectangle sums (linear in x) and passed in. On-device: row sums of
     squares (DVE STT bypass*self trick), Gram G0 = q @ k^T via DMA-transposed
     tiles accumulated on PE.
  3. Permutation matrix P from host-provided ranks; attention computed as
     dense 128x128 with block-diagonal masking.
  4. out = W_proj(mod + attn) streamed over L; the up-projection is folded
     into W_proj on the host (W_pu = W_proj @ W_up) saving a matmul per tile.

Outputs per core: out1 (C,L) fp32 and stats (C,4) fp32 [v0, rank, -, ssq_q].
Host assembles the qv_cache output (broadcast of a length-128 vector) in numpy.
"""

import sys

sys.path.insert(0, "/opt/trn_rl_repo")

import numpy as np
import ml_dtypes
from contextlib import ExitStack

import concourse.bass as bass
import concourse.bacc as bacc
import concourse.tile as tile

from concourse import mybir
from concourse.bass_utils import run_bass_kernel_spmd

F32 = mybir.dt.float32
BF16 = mybir.dt.bfloat16
FP8 = mybir.dt.float8e4
BD = ml_dtypes.bfloat16
F8 = ml_dtypes.float8_e4m3

C = 128
H = 128
W = 128
L = H * W
B = 8
NCORES = 8
GROUP_SIZES = [16, 32, 32, 48]

CHUNK_ROWS = 16
NCH = H // CHUNK_ROWS
ROWSPAN = CHUNK_ROWS + 2          # chunk rows + 1 halo row each side
GAPW = W + 2                      # image row + 2 zero gap cols
XBCOLS = 2 + ROWSPAN * GAPW      # leading 2-col zero gap keeps rows 4B aligned

PE_TAPS = [(dy, dx) for dy in (-1, 0, 1) for dx in (-1, 0, 1)]
W8SCALE = 2.0 ** 13               # fp8 weight pre-scale (descaled on PSUM copy)
VPROWS = H + 2                    # full-plane v_pre gap buffer (1 pad row each side)
VPCOLS = 2 + VPROWS * GAPW

P2_TILE = 512
N_P2 = L // P2_TILE

ADD = mybir.AluOpType.add
MULT = mybir.AluOpType.mult
AF = mybir.ActivationFunctionType
DR = mybir.MatmulPerfMode.DoubleRow


def view3(t, off, rows, rowstride, w):
    """Strided 3D view into a 2D sbuf tile: (partitions, rows, w)."""
    return bass.AP(tensor=t.tensor, offset=t.offset + off, ap=[t.ap[0], [rowstride, rows], [1, w]])


def view4(t, off, dpair, rows, rowstride, w):
    """(partitions, 2, rows, w) pair view for DoubleRow matmuls."""
    return bass.AP(tensor=t.tensor, offset=t.offset + off,
                   ap=[t.ap[0], [dpair, 2], [rowstride, rows], [1, w]])


def build_bass():
    nc = bacc.Bacc()
    _build_body(nc)
    nc.compile()
    return nc


def _build_body(nc):
    # ---- dram parameters -------------------------------------------------
    # x arrives from the host already in the padded gap layout (one zero pad
    # row top+bottom, 2 zero gap cols per row) so every chunk DMA is one
    # contiguous 2340B-per-partition descriptor instead of 18 strided 256B ones
    xbf_h = nc.declare_dram_parameter("xbf", [C, (H + 2) * GAPW], BF16, isOutput=False)
    x8_h = nc.declare_dram_parameter("x8", [C, (H + 2) * GAPW], FP8, isOutput=False)
    NBF = 12 * C + 64  # wv-taps*9 wgate wproj ident (C cols each) + wdown(64)
    NF32 = 4 * C + 15  # mask negb iotar rankrow + [bgate bup bpu temp rank iotac] + wdw_v*9
    packbf_h = nc.declare_dram_parameter("packbf", [C, NBF], BF16, isOutput=False)
    pack8_h = nc.declare_dram_parameter("pack8", [C, 20 * C], FP8, isOutput=False)
    wpu_h = nc.declare_dram_parameter("wpu_t", [64, C], BF16, isOutput=False)
    packf_h = nc.declare_dram_parameter("packf", [C, NF32], F32, isOutput=False)
    rs_h = nc.declare_dram_parameter("rs", [C, 2], F32, isOutput=False)
    bdown_h = nc.declare_dram_parameter("bdown", [64, 1], F32, isOutput=False)
    out1_h = nc.declare_dram_parameter("out1", [C, L], F32, isOutput=True)
    stats_h = nc.declare_dram_parameter("stats", [C, 4], F32, isOutput=True)

    with tile.TileContext(nc) as tc, ExitStack() as ctx:
        singles = ctx.enter_context(tc.tile_pool(name="singles", bufs=1))
        stat = ctx.enter_context(tc.tile_pool(name="stat", bufs=1))
        dwbig = ctx.enter_context(tc.tile_pool(name="dwbig", bufs=1))

        # ---- constants to SBUF: packed DMAs on separate queues ----------
        s_packbf = singles.tile([C, NBF], BF16, tag="s_packbf", name="s_packbf")
        nc.scalar.dma_start(out=s_packbf[:, :], in_=packbf_h[:, :])
        s_pack8 = singles.tile([C, 20 * C], FP8, tag="s_pack8", name="s_pack8")
        nc.scalar.dma_start(out=s_pack8[:, :], in_=pack8_h[:, :])
        s_packf = singles.tile([C, NF32], F32, tag="s_packf", name="s_packf")
        s_rs = singles.tile([C, 2], F32, tag="s_rs", name="s_rs")
        s_wpu = singles.tile([64, C], BF16, tag="s_wpu", name="s_wpu")
        s_bdown = singles.tile([64, 1], F32, tag="s_bdown", name="s_bdown")
        nc.scalar.dma_start(out=s_bdown[:, :], in_=bdown_h[:, :])

        def bfcol(i):
            return s_packbf[:, i * C:(i + 1) * C]

        s_wv_taps = [bfcol(i) for i in range(9)]
        s_wgate = bfcol(9)
        s_wproj = bfcol(10)
        s_ident = bfcol(11)
        s_wdown = s_packbf[:, 12 * C:12 * C + 64]   # W_down^T : [C, 64]
        s_mask = s_packf[:, 0:C]
        s_negb = s_packf[:, C:2 * C]
        s_iotar = s_packf[:, 2 * C:3 * C]
        s_rankrow = s_packf[:, 3 * C:4 * C]
        s_bgate = s_packf[:, 4 * C + 0:4 * C + 1]
        s_bpu = s_packf[:, 4 * C + 2:4 * C + 3]
        s_temp = s_packf[:, 4 * C + 3:4 * C + 4]
        s_rank = s_packf[:, 4 * C + 4:4 * C + 5]
        s_iotac = s_packf[:, 4 * C + 5:4 * C + 6]

        # ---- persistent state -------------------------------------------
        dw = [dwbig.tile([C, L], BF16, tag=f"dw{p}", name=f"dw{p}") for p in range(3)]
        dw8 = dwbig.tile([C, 2, L], FP8, tag="dw8", name="dw8")  # q,k scaled 2^5 for pass-2 pair
        gram_acc = stat.tile([C, C], F32, tag="gram", name="gram")
        sqsums = stat.tile([C, 2, NCH], F32, tag="sqsums", name="sqsums")  # q/k row-sumsq partials
        spack = stat.tile([C, 16], F32, tag="spack", name="spack")

        # double-buffered gap-layout x chunks (bf16 for v, fp8 for q/k): the
        # interior [2:] is fully DMA-overwritten every chunk; only the 2-col
        # lead gap needs zeroing, once
        xbb_bufs, xb8_bufs = [], []
        for i in range(2):
            xbb_bufs.append(stat.tile([C, XBCOLS], BF16, tag=f"xbb{i}", name=f"xbb{i}"))
            xb8_bufs.append(stat.tile([C, XBCOLS], FP8, tag=f"xb8{i}", name=f"xb8{i}"))
        for i in range(2):
            nc.vector.memset(xbb_bufs[i][:, 0:2], 0.0)
            nc.vector.memset(xb8_bufs[i][:, 0:2], 0.0)
        nc.vector.memset(gram_acc[:, :], 0.0)

        NSL = CHUNK_ROWS // 4
        with ExitStack() as p1:
            trp = p1.enter_context(tc.tile_pool(name="trp", bufs=2))
            scrp = p1.enter_context(tc.tile_pool(name="scrp", bufs=2))
            psdw = p1.enter_context(tc.tile_pool(name="psdw", bufs=6, space="PSUM"))
            psg = p1.enter_context(tc.tile_pool(name="psg", bufs=2, space="PSUM"))

            def issue_x_prefetch(ch):
                r0 = ch * CHUNK_ROWS
                nc.sync.dma_start(
                    out=xbb_bufs[ch % 2][:, 2:XBCOLS],
                    in_=xbf_h[:, r0 * GAPW:(r0 + ROWSPAN) * GAPW])
                nc.gpsimd.dma_start(
                    out=xb8_bufs[ch % 2][:, 2:XBCOLS],
                    in_=x8_h[:, r0 * GAPW:(r0 + ROWSPAN) * GAPW])

            def issue_gram(trq, trk):
                # Gram partial: G0 += q_ch @ k_ch^T (runs a chunk behind so
                # the PE never waits on the transposes)
                gps = psg.tile([C, C], F32, tag="gps", name="gps")
                for j in range(CHUNK_ROWS):
                    nc.tensor.matmul(gps[:, :], trq[:, j, :], trk[:, j, :],
                                     start=(j == 0), stop=(j == CHUNK_ROWS - 1))
                nc.vector.tensor_add(gram_acc[:, :], gram_acc[:, :], gps[:, :])

            # chunk-0 x before the (late-needed) constant DMAs for fast start;
            # the small constants ride the scalar queue so gpsimd only carries
            # the x8 chunk stream
            issue_x_prefetch(0)
            nc.scalar.dma_start(out=s_packf[:, :], in_=packf_h[:, :])
            nc.scalar.dma_start(out=s_rs[:, :], in_=rs_h[:, :])
            nc.scalar.dma_start(out=s_wpu[:, :], in_=wpu_h[:, :])

            # hoist the permutation-matrix build (depends only on constants)
            # so the small phase after the chunk loop is shorter. P and P^T
            # are built directly on DVE from rank/iota compares - no PE
            # transpose needed.
            ptf = stat.tile([C, C], F32, tag="ptf", name="ptf")
            nc.vector.tensor_scalar(out=ptf[:, :], in0=s_iotar[:, :], scalar1=s_rank[:, :],
                                    scalar2=None, op0=mybir.AluOpType.is_equal)
            pt_bf = stat.tile([C, C], BF16, tag="pt_bf", name="pt_bf")
            nc.vector.tensor_copy(out=pt_bf[:, :], in_=ptf[:, :])
            p_bf = stat.tile([C, C], BF16, tag="p_bf", name="p_bf")
            nc.vector.tensor_scalar(out=p_bf[:, :], in0=s_rankrow[:, :],
                                    scalar1=s_iotac[:, :], scalar2=None,
                                    op0=mybir.AluOpType.is_equal)

            prev_tr = None
            for ch in range(NCH):
                r0 = ch * CHUNK_ROWS
                if ch + 1 < NCH:
                    issue_x_prefetch(ch + 1)
                xbb = xbb_bufs[ch % 2]
                xb8 = xb8_bufs[ch % 2]

                tr_tiles = {}
                # ---- v: 9 folded bf16 taps (accuracy-critical plane)
                for sl in range(NSL):
                    j0 = sl * 4
                    pd = psdw.tile([C, 4 * W], F32, tag="psdw", name="psdw")
                    for i, (dy, dx) in enumerate(PE_TAPS):
                        rhs = view3(xbb, 2 + (1 + j0 + dy) * GAPW + dx, 4, GAPW, W)
                        nc.tensor.matmul(pd[:, :], s_wv_taps[i][:, :], rhs,
                                         start=(i == 0), stop=(i == 8))
                    nc.scalar.activation(
                        out=dw[2][:, (r0 + j0) * W:(r0 + j0 + 4) * W], in_=pd[:, :],
                        func=AF.Copy)

                for p in (0, 1):
                    dwbuf = dw[p]
                    for sl in range(NSL):
                        j0 = sl * 4
                        pd = psdw.tile([C, 4 * W], F32, tag="psdw", name="psdw")
                        # q/k: 5 DoubleRow pairs (9 taps + 1 zero pad)
                        for i in range(5):
                            ta = PE_TAPS[2 * i]
                            if i < 4:
                                tb = PE_TAPS[2 * i + 1]
                                d = (tb[0] - ta[0]) * GAPW + (tb[1] - ta[1])
                            else:
                                d = 1  # zero-weighted pad tap
                            lhsT = view3(s_pack8, (p * 10 + 2 * i) * C, 2, C, C)
                            rhs = view4(xb8, 2 + (1 + j0 + ta[0]) * GAPW + ta[1],
                                        d, 4, GAPW, W)
                            nc.tensor.matmul(pd[:, :], lhsT, rhs,
                                             start=(i == 0), stop=(i == 4),
                                             perf_mode=DR)
                        nc.scalar.activation(
                            out=dwbuf[:, (r0 + j0) * W:(r0 + j0 + 4) * W], in_=pd[:, :],
                            func=AF.Copy, scale=1.0 / W8SCALE)

                    # ---- fp8 copy, row sum-of-squares, transpose for q,k
                    chsl = dwbuf[:, r0 * W:(r0 + CHUNK_ROWS) * W]
                    nc.vector.tensor_scalar_mul(
                        out=dw8[:, p, r0 * W:(r0 + CHUNK_ROWS) * W], in0=chsl,
                        scalar1=2.0 ** 5)
                    scr = scrp.tile([C, CHUNK_ROWS * W], BF16, tag="sqscr", name="sqscr")
                    nc.vector.scalar_tensor_tensor(
                        out=scr[:, :], in0=chsl, scalar=0.0, in1=chsl,
                        op0=mybir.AluOpType.bypass, op1=MULT,
                        accum_out=sqsums[:, p, ch:ch + 1])
                    tr = trp.tile([C, CHUNK_ROWS, W], BF16, tag=f"tr{p}", name=f"tr{p}")
                    nc.sync.dma_start_transpose(out=tr[:, :, :], in_=chsl)
                    tr_tiles[p] = tr

                # lagged by one chunk: gram (PE) for ch-1 so the PE never
                # waits on the transposes
                if ch > 0:
                    issue_gram(prev_tr[0], prev_tr[1])
                prev_tr = tr_tiles

            issue_gram(prev_tr[0], prev_tr[1])

        # ================= small-matrix phase ============================
        with ExitStack() as sm:
            smp = sm.enter_context(tc.tile_pool(name="smp", bufs=1))
            pss_ctx = ExitStack()
            pss = pss_ctx.enter_context(tc.tile_pool(name="pss", bufs=2, space="PSUM"))

            # rnorm_q / rnorm_k; PDqT / PDkT
            pd8 = smp.tile([C, 2 * C], FP8, tag="pd8", name="pd8")
            pd_bf = []
            for pi in range(2):
                nc.vector.tensor_reduce(out=spack[:, 3 + pi:4 + pi], in_=sqsums[:, pi, :],
                                        axis=mybir.AxisListType.X, op=ADD)
                nc.scalar.activation(out=spack[:, 5 + pi:6 + pi], in_=spack[:, 3 + pi:4 + pi],
                                     func=AF.Sqrt)
                nc.vector.tensor_scalar_max(out=spack[:, 5 + pi:6 + pi],
                                            in0=spack[:, 5 + pi:6 + pi], scalar1=1e-12)
                nc.vector.reciprocal(out=spack[:, 5 + pi:6 + pi], in_=spack[:, 5 + pi:6 + pi])
                t = smp.tile([C, C], BF16, tag=f"pd{pi}", name=f"pd{pi}")
                nc.vector.tensor_scalar_mul(out=t[:, :], in0=ptf[:, :],
                                            scalar1=spack[:, 5 + pi:6 + pi])
                pd_bf.append(t)
                # fp8 copy (x 2^-5 to undo the dw8 scale) for the pass-2 pair
                nc.vector.tensor_scalar_mul(out=spack[:, 13:14],
                                            in0=spack[:, 5 + pi:6 + pi],
                                            scalar1=2.0 ** -5)
                nc.vector.tensor_scalar_mul(out=pd8[:, pi * C:(pi + 1) * C],
                                            in0=ptf[:, :], scalar1=spack[:, 13:14])

            # v0 = (qsum*rnq + ksum*rnk)/L  (qsum/ksum exact from host)
            nc.vector.tensor_mul(spack[:, 8:9], s_rs[:, 0:1], spack[:, 5:6])
            nc.vector.tensor_mul(spack[:, 9:10], s_rs[:, 1:2], spack[:, 6:7])
            nc.vector.tensor_add(spack[:, 8:9], spack[:, 8:9], spack[:, 9:10])
            nc.vector.tensor_scalar_mul(out=spack[:, 8:9], in0=spack[:, 8:9], scalar1=1.0 / L)
            sout = smp.tile([C, 4], F32, tag="sout", name="sout")
            nc.vector.tensor_copy(out=sout[:, 0:1], in_=spack[:, 8:9])   # v0
            nc.vector.tensor_copy(out=sout[:, 1:2], in_=s_rank[:, :])    # rank echo
            nc.vector.tensor_copy(out=sout[:, 2:3], in_=s_rs[:, 0:1])    # qsum echo
            nc.vector.tensor_copy(out=sout[:, 3:4], in_=spack[:, 3:4])   # ssq_q
            nc.sync.dma_start(out=stats_h[:, :], in_=sout[:, :])

            # Gp = (P Dq) G0 (Dk P^T)
            g0_bf = smp.tile([C, C], BF16, tag="g0bf", name="g0bf")
            nc.vector.tensor_copy(out=g0_bf[:, :], in_=gram_acc[:, :])
            t1ps = pss.tile([C, C], F32, tag="psf", name="psf")
            nc.tensor.matmul(t1ps[:, :], pd_bf[0][:, :], g0_bf[:, :], start=True, stop=True)
            t1_bf = smp.tile([C, C], BF16, tag="t1bf", name="t1bf")
            nc.scalar.copy(out=t1_bf[:, :], in_=t1ps[:, :])
            t1tps = pss.tile([C, C], BF16, tag="psbf", name="psbf")
            nc.tensor.transpose(t1tps[:, :], t1_bf[:, :], s_ident[:, :])
            t1t_bf = smp.tile([C, C], BF16, tag="t1tbf", name="t1tbf")
            nc.scalar.copy(out=t1t_bf[:, :], in_=t1tps[:, :])
            gpps = pss.tile([C, C], F32, tag="psf", name="psf")
            nc.tensor.matmul(gpps[:, :], t1t_bf[:, :], pd_bf[1][:, :], start=True, stop=True)

            # masked block-diagonal softmax (rows = rank space)
            xsm = smp.tile([C, C], F32, tag="xsm", name="xsm")
            nc.vector.tensor_scalar_mul(out=xsm[:, :], in0=gpps[:, :], scalar1=s_temp[:, :])
            nc.vector.tensor_mul(xsm[:, :], xsm[:, :], s_mask[:, :])
            nc.vector.tensor_add(xsm[:, :], xsm[:, :], s_negb[:, :])
            nc.vector.tensor_reduce(out=spack[:, 11:12], in_=xsm[:, :],
                                    axis=mybir.AxisListType.X, op=mybir.AluOpType.max)
            nc.vector.tensor_scalar_sub(out=xsm[:, :], in0=xsm[:, :], scalar1=spack[:, 11:12])
            nc.scalar.activation(out=xsm[:, :], in_=xsm[:, :], func=AF.Exp,
                                 accum_out=spack[:, 12:13])
            nc.vector.reciprocal(out=spack[:, 12:13], in_=spack[:, 12:13])
            a_bf = smp.tile([C, C], BF16, tag="a_bf", name="a_bf")
            nc.vector.tensor_scalar_mul(out=a_bf[:, :], in0=xsm[:, :], scalar1=spack[:, 12:13])

            # W_v = P^T A^T  (lhsT for the attention matmul over v)
            atps = pss.tile([C, C], BF16, tag="psbf", name="psbf")
            nc.tensor.transpose(atps[:, :], a_bf[:, :], s_ident[:, :])
            at_bf = smp.tile([C, C], BF16, tag="at_bf", name="at_bf")
            nc.scalar.copy(out=at_bf[:, :], in_=atps[:, :])
            wvps = pss.tile([C, C], F32, tag="psf", name="psf")
            nc.tensor.matmul(wvps[:, :], p_bf[:, :], at_bf[:, :], start=True, stop=True)
            wv_bf = smp.tile([C, C], BF16, tag="wv_bf", name="wv_bf")
            nc.scalar.copy(out=wv_bf[:, :], in_=wvps[:, :])

            # wpa = (W_proj A P)^T = P^T A^T W_proj^T
            m1ps = pss.tile([C, C], F32, tag="psf", name="psf2")
            nc.tensor.matmul(m1ps[:, :], a_bf[:, :], s_wproj[:, :], start=True, stop=True)
            m1_bf = smp.tile([C, C], BF16, tag="m1_bf", name="m1_bf")
            nc.scalar.copy(out=m1_bf[:, :], in_=m1ps[:, :])
            wpaps = pss.tile([C, C], F32, tag="psf", name="psf3")
            nc.tensor.matmul(wpaps[:, :], p_bf[:, :], m1_bf[:, :], start=True, stop=True)
            wpa_bf = smp.tile([C, C], BF16, tag="wpa_bf", name="wpa_bf")
            nc.scalar.copy(out=wpa_bf[:, :], in_=wpaps[:, :])

            pss_ctx.close()

            # ============== pass 2: streamed output ======================
            with ExitStack() as p2x:
                p2 = p2x.enter_context(tc.tile_pool(name="p2", bufs=3))
                psS = p2x.enter_context(tc.tile_pool(name="psS", bufs=2, space="PSUM"))
                psG2 = p2x.enter_context(tc.tile_pool(name="psG2", bufs=2, space="PSUM"))
                psD = p2x.enter_context(tc.tile_pool(name="psD", bufs=1, space="PSUM"))
                psO = p2x.enter_context(tc.tile_pool(name="psO", bufs=2, space="PSUM"))

                for t in range(N_P2):
                    c0 = t * P2_TILE
                    q_sl = dw[0][:, c0:c0 + P2_TILE]
                    k_sl = dw[1][:, c0:c0 + P2_TILE]
                    v_sl = dw[2][:, c0:c0 + P2_TILE]

                    # sxy = (A P) v + (P Dq) q + (P Dk) k; the q,k terms run
                    # as one fp8 DoubleRow pair over the dw8 copies
                    ps = psS.tile([C, P2_TILE], F32, tag="ps", name="ps")
                    nc.tensor.matmul(ps[:, :], wv_bf[:, :], v_sl, start=True, stop=False)
                    nc.tensor.matmul(ps[:, :], view3(pd8, 0, 2, C, C),
                                     view3(dw8, c0, 2, L, P2_TILE),
                                     start=False, stop=True, perf_mode=DR)
                    sxy_sb = p2.tile([C, P2_TILE], BF16, tag="sxy", name="sxy_sb")
                    nc.vector.tensor_copy(out=sxy_sb[:, :], in_=ps[:, :])

                    pg = psG2.tile([C, P2_TILE], F32, tag="pg", name="pg")
                    nc.tensor.matmul(pg[:, :], s_wgate[:, :], sxy_sb[:, :], start=True, stop=True)
                    gat_sb = p2.tile([C, P2_TILE], BF16, tag="gat", name="gat_sb")
                    nc.scalar.activation(out=gat_sb[:, :], in_=pg[:, :], func=AF.Gelu,
                                         bias=s_bgate[:, :], scale=1.0)
                    gated_sb = p2.tile([C, P2_TILE], BF16, tag="gated", name="gated_sb")
                    nc.vector.tensor_mul(gated_sb[:, :], gat_sb[:, :], sxy_sb[:, :])

                    pdn = psD.tile([64, P2_TILE], F32, tag="pd2", name="pdn")
                    nc.tensor.matmul(pdn[:, :], s_wdown[:, :], gated_sb[:, :],
                                     start=True, stop=True)
                    d_sb = p2.tile([64, P2_TILE], BF16, tag="dsb", name="d_sb")
                    nc.scalar.activation(out=d_sb[:, :], in_=pdn[:, :], func=AF.Identity,
                                         bias=s_bdown[:, :], scale=1.0)

                    # out = (W_proj A P) v + (W_proj W_up) d  (+ W_proj b_up bias)
                    po = psO.tile([C, P2_TILE], F32, tag="po", name="po")
                    nc.tensor.matmul(po[:, :], wpa_bf[:, :], v_sl, start=True, stop=False)
                    nc.tensor.matmul(po[:, :], s_wpu[:, :], d_sb[:, :],
                                     start=False, stop=True)
                    outf = p2.tile([C, P2_TILE], F32, tag="outf", name="outf")
                    nc.scalar.activation(out=outf[:, :], in_=po[:, :], func=AF.Identity,
                                         bias=s_bpu[:, :], scale=1.0)
                    nc.sync.dma_start(out=out1_h[:, c0:c0 + P2_TILE], in_=outf[:, :])


_NC_CACHE = None


def _get_nc():
    global _NC_CACHE
    if _NC_CACHE is None:
        _NC_CACHE = build_bass()
    return _NC_CACHE


def _host_inputs(x, temperature, w_qkv, w_dw, w_proj, w_gate, b_gate,
                 w_down, b_down, w_up, b_up):
    f = np.float32
    x = np.asarray(x, f).reshape(B, C, L)
    w_qkv = np.asarray(w_qkv, f)
    w_dw = np.asarray(w_dw, f)
    temperature = np.asarray(temperature, f)

    shared = {}
    packbf = np.zeros((C, 12 * C + 64), np.float32)
    for ti, (dy, dx) in enumerate(PE_TAPS):
        wt = w_dw[2 * C:3 * C, 0, dy + 1, dx + 1]
        packbf[:, ti * C:(ti + 1) * C] = (w_qkv[2 * C:3 * C, :] * wt[:, None]).T
    packbf[:, 9 * C:10 * C] = np.asarray(w_gate, f).T
    packbf[:, 10 * C:11 * C] = np.asarray(w_proj, f).T
    packbf[:, 11 * C:12 * C] = np.eye(C, dtype=f)
    packbf[:, 12 * C:12 * C + 64] = np.asarray(w_down, f).T
    shared["packbf"] = packbf.astype(BD)
    wpu = np.asarray(w_proj, f) @ np.asarray(w_up, f)          # (C, 64)
    shared["wpu_t"] = np.ascontiguousarray(wpu.T).astype(BD)   # (64, C)

    pack8 = np.zeros((C, 20 * C), np.float32)
    for p in range(2):
        for ti, (dy, dx) in enumerate(PE_TAPS):
            wt = w_dw[p * C:(p + 1) * C, 0, dy + 1, dx + 1]
            pack8[:, (p * 10 + ti) * C:(p * 10 + ti + 1) * C] = \
                (w_qkv[p * C:(p + 1) * C, :] * wt[:, None]).T * W8SCALE
    shared["pack8"] = pack8.astype(F8)

    # exact channel means of dwconv(Wq x) via rectangle sums (linear in x),
    # and exact per-image row sums for q and k planes
    xr = x.reshape(B, C, H, W).astype(np.float64)
    wq = w_qkv[:C, :].astype(np.float64)
    wk = w_qkv[C:2 * C, :].astype(np.float64)
    wdw_q = w_dw[:C, 0].astype(np.float64)
    wdw_k = w_dw[C:2 * C, 0].astype(np.float64)
    mean = np.zeros(C, np.float64)
    rs_q = np.zeros((B, C), np.float64)
    rs_k = np.zeros((B, C), np.float64)
    for dy in (-1, 0, 1):
        for dx in (-1, 0, 1):
            y0, y1 = max(0, dy), min(H - 1, H - 1 + dy)
            x0, x1 = max(0, dx), min(W - 1, W - 1 + dx)
            rect_b = xr[:, :, y0:y1 + 1, x0:x1 + 1].sum(axis=(2, 3))   # (B, C)
            rect = rect_b.sum(axis=0)
            mean += wdw_q[:, dy + 1, dx + 1] * (wq @ rect)
            rs_q += wdw_q[None, :, dy + 1, dx + 1] * (rect_b @ wq.T)
            rs_k += wdw_k[None, :, dy + 1, dx + 1] * (rect_b @ wk.T)
    mean /= float(B * L)
    idx_order = np.argsort(-mean, kind="stable")
    rank = np.empty(C, np.int64)
    rank[idx_order] = np.arange(C)

    gid = np.zeros(C, np.int64)
    s = 0
    for gi, g in enumerate(GROUP_SIZES):
        gid[s:s + g] = gi
        s += g
    same = (gid[:, None] == gid[None, :])
    idx = np.arange(C)
    packf = np.zeros((C, 4 * C + 15), f)
    packf[:, 0:C] = same.astype(f)
    packf[:, C:2 * C] = np.where(same, 0.0, -30000.0)
    packf[:, 2 * C:3 * C] = np.tile(idx[None, :], (C, 1))
    packf[:, 3 * C:4 * C] = np.tile(rank[None, :], (C, 1)).astype(f)
    packf[:, 4 * C + 0] = np.asarray(b_gate, f)
    packf[:, 4 * C + 1] = np.asarray(b_up, f)
    packf[:, 4 * C + 2] = np.asarray(w_proj, f) @ np.asarray(b_up, f)
    packf[:, 4 * C + 3] = temperature[gid, 0, 0]
    packf[:, 4 * C + 4] = rank.astype(f)
    packf[:, 4 * C + 5] = idx.astype(f)
    shared["packf"] = packf
    shared["bdown"] = np.asarray(b_down, f).reshape(64, 1)

    xg = np.zeros((B, C, H + 2, GAPW), np.float32)
    xg[:, :, 1:H + 1, :W] = x.reshape(B, C, H, W)
    xg = xg.reshape(B, C, (H + 2) * GAPW)
    in_maps = []
    for i in range(NCORES):
        m = dict(shared)
        m["xbf"] = xg[i].astype(BD)
        m["x8"] = xg[i].astype(F8)
        m["rs"] = np.stack([rs_q[i], rs_k[i]], axis=1).astype(f)
        in_maps.append(m)
    return in_maps


def _assemble(results):
    out = np.zeros((B, C, H, W), np.float32)
    cache = np.zeros((B, C, H, W), np.float32)
    for i in range(NCORES):
        out[i] = np.asarray(results[i]["out1"], np.float32).reshape(C, H, W)
        st = np.asarray(results[i]["stats"], np.float32)
        v0 = st[:, 0]
        rank = np.rint(st[:, 1]).astype(np.int64)
        idx_dev = np.argsort(rank)
        mt = v0[idx_dev]
        s = 0
        gms = []
        for g in GROUP_SIZES:
            gm = mt[s:s + g]
            s += g
            rep = max(1, C // g)
            gm = np.tile(gm, rep)
            if gm.shape[0] >= C:
                gm = gm[:C]
            else:
                gm = np.pad(gm, (0, C - gm.shape[0]))
            gms.append(gm)
        acc = np.mean(np.stack(gms, 0), 0)
        cache[i] = np.broadcast_to((acc * 0.9)[:, None, None], (C, H, W))
    return out, cache


def kernel(**inputs):
    nc = _get_nc()
    in_maps = _host_inputs(**inputs)
    res = run_bass_kernel_spmd(nc, in_maps, list(range(NCORES)))
    return _assemble(res.results)


if __name__ == "__main__":
    rng = np.random.default_rng(0)
    dummy = {
        "x": rng.standard_normal((B, C, H, W), dtype=np.float32),
        "temperature": np.ones((4, 1, 1), np.float32),
        "w_qkv": rng.standard_normal((3 * C, C), dtype=np.float32) * 0.02,
        "w_dw": rng.standard_normal((3 * C, 1, 3, 3), dtype=np.float32) * 0.02,
        "w_proj": rng.standard_normal((C, C), dtype=np.float32) * 0.02,
        "w_gate": rng.standard_normal((C, C), dtype=np.float32) * 0.02,
        "b_gate": np.zeros(C, np.float32),
        "b_down": np.zeros(C // 2, np.float32),
        "w_down": rng.standard_normal((C // 2, C), dtype=np.float32) * 0.02,
        "w_up": rng.standard_normal((C, C // 2), dtype=np.float32) * 0.02,
        "b_up": np.zeros(C, np.float32),
    }
    o, c = kernel(**dummy)
    print("out", o.shape, o.dtype, "cache", c.shape, c.dtype)


# revision 28
# speedup vs baseline: 1.1854x; 1.1854x over previous
"""Trainium2 Bass kernel for nn_Attention_34806414967022 (sparse channel attention).

Data-parallel over batch: 8 batch images -> 8 NeuronCores, one image each.

Per-core pipeline (image = (C=128, H=128, W=128), L = H*W = 16384):
  1. Depthwise-3x3(Conv1x1(x)) planes, fully folded into PE matmuls
     (W_eff[tap] = diag(w_dw[:,tap]) @ W_qkv applied to shifted x views):
       - q,k planes: fp8(e4m3) weights+activations, taps fused in pairs via
         MatmulPerfMode.DoubleRow (2 taps per instruction at 2x rate).
         Weights pre-scaled by 2^13 on host, descaled in the PSUM evacuation.
       - v plane: bf16 (accuracy-critical: attn@v feeds the output directly).
     Shift safety: x chunks live in a gap layout (row stride 130, two zero gap
     cols) so +-1 column shifts read zeros at row edges. x arrives pre-cast
     from the host (bf16 + fp8 copies) and is DMAed straight into the gap
     layout - no on-device cast.
  2. Channel stats: row sums of q,k are computed EXACTLY on the host via
     rectangle sums (linear in x) and passed in. On-device: row sums of
     squares (DVE STT bypass*self trick), Gram G0 = q @ k^T via DMA-transposed
     tiles accumulated on PE.
  3. Permutation matrix P from host-provided ranks; attention computed as
     dense 128x128 with block-diagonal masking.
  4. out = W_proj(mod + attn) streamed over L; the up-projection is folded
     into W_proj on the host (W_pu = W_proj @ W_up) saving a matmul per tile.

Outputs per core: out1 (C,L) fp32 and stats (C,4) fp32 [v0, rank, -, ssq_q].
Host assembles the qv_cache output (broadcast of a length-128 vector) in numpy.
"""

import sys

sys.path.insert(0, "/opt/trn_rl_repo")

import numpy as np
import ml_dtypes
from contextlib import ExitStack

import concourse.bass as bass
import concourse.bacc as bacc
import concourse.tile as tile

from concourse import mybir
from concourse.bass_utils import run_bass_kernel_spmd

F32 = mybir.dt.float32
BF16 = mybir.dt.bfloat16
FP8 = mybir.dt.float8e4
BD = ml_dtypes.bfloat16
F8 = ml_dtypes.float8_e4m3

C = 128
H = 128
W = 128
L = H * W
B = 8
NCORES = 8
GROUP_SIZES = [16, 32, 32, 48]

CHUNK_ROWS = 16
NCH = H // CHUNK_ROWS
ROWSPAN = CHUNK_ROWS + 2          # chunk rows + 1 halo row each side
GAPW = W + 2                      # image row + 2 zero gap cols
XBCOLS = 2 + ROWSPAN * GAPW      # leading 2-col zero gap keeps rows 4B aligned

PE_TAPS = [(dy, dx) for dy in (-1, 0, 1) for dx in (-1, 0, 1)]
W8SCALE = 2.0 ** 13               # fp8 weight pre-scale (descaled on PSUM copy)
VPROWS = H + 2                    # full-plane v_pre gap buffer (1 pad row each side)
VPCOLS = 2 + VPROWS * GAPW

P2_TILE = 512
N_P2 = L // P2_TILE

ADD = mybir.AluOpType.add
MULT = mybir.AluOpType.mult
AF = mybir.ActivationFunctionType
DR = mybir.MatmulPerfMode.DoubleRow


def view3(t, off, rows, rowstride, w):
    """Strided 3D view into a 2D sbuf tile: (partitions, rows, w)."""
    return bass.AP(tensor=t.tensor, offset=t.offset + off, ap=[t.ap[0], [rowstride, rows], [1, w]])


def view4(t, off, dpair, rows, rowstride, w):
    """(partitions, 2, rows, w) pair view for DoubleRow matmuls."""
    return bass.AP(tensor=t.tensor, offset=t.offset + off,
                   ap=[t.ap[0], [dpair, 2], [rowstride, rows], [1, w]])


def build_bass():
    nc = bacc.Bacc()
    _build_body(nc)
    nc.compile()
    return nc


def _build_body(nc):
    # ---- dram parameters -------------------------------------------------
    # x arrives from the host already in the padded gap layout (one zero pad
    # row top+bottom, 2 zero gap cols per row) so every chunk DMA is one
    # contiguous 2340B-per-partition descriptor instead of 18 strided 256B ones
    xbf_h = nc.declare_dram_parameter("xbf", [C, (H + 2) * GAPW], BF16, isOutput=False)
    x8_h = nc.declare_dram_parameter("x8", [C, (H + 2) * GAPW], FP8, isOutput=False)
    NBF = 12 * C + 64  # wv-taps*9 wgate wproj ident (C cols each) + wdown(64)
    NF32 = 4 * C + 15  # mask negb iotar rankrow + [bgate bup bpu temp rank iotac] + wdw_v*9
    packbf_h = nc.declare_dram_parameter("packbf", [C, NBF], BF16, isOutput=False)
    pack8_h = nc.declare_dram_parameter("pack8", [C, 20 * C], FP8, isOutput=False)
    wpu_h = nc.declare_dram_parameter("wpu_t", [64, C], BF16, isOutput=False)
    packf_h = nc.declare_dram_parameter("packf", [C, NF32], F32, isOutput=False)
    rs_h = nc.declare_dram_parameter("rs", [C, 2], F32, isOutput=False)
    bdown_h = nc.declare_dram_parameter("bdown", [64, 1], F32, isOutput=False)
    out1_h = nc.declare_dram_parameter("out1", [C, L], F32, isOutput=True)
    stats_h = nc.declare_dram_parameter("stats", [C, 4], F32, isOutput=True)

    with tile.TileContext(nc) as tc, ExitStack() as ctx:
        singles = ctx.enter_context(tc.tile_pool(name="singles", bufs=1))
        stat = ctx.enter_context(tc.tile_pool(name="stat", bufs=1))
        dwbig = ctx.enter_context(tc.tile_pool(name="dwbig", bufs=1))

        # ---- constants to SBUF: packed DMAs on separate queues ----------
        s_packbf = singles.tile([C, NBF], BF16, tag="s_packbf", name="s_packbf")
        nc.scalar.dma_start(out=s_packbf[:, :], in_=packbf_h[:, :])
        s_pack8 = singles.tile([C, 20 * C], FP8, tag="s_pack8", name="s_pack8")
        nc.scalar.dma_start(out=s_pack8[:, :], in_=pack8_h[:, :])
        s_packf = singles.tile([C, NF32], F32, tag="s_packf", name="s_packf")
        s_rs = singles.tile([C, 2], F32, tag="s_rs", name="s_rs")
        s_wpu = singles.tile([64, C], BF16, tag="s_wpu", name="s_wpu")
        s_bdown = singles.tile([64, 1], F32, tag="s_bdown", name="s_bdown")
        nc.scalar.dma_start(out=s_bdown[:, :], in_=bdown_h[:, :])

        def bfcol(i):
            return s_packbf[:, i * C:(i + 1) * C]

        s_wv_taps = [bfcol(i) for i in range(9)]
        s_wgate = bfcol(9)
        s_wproj = bfcol(10)
        s_ident = bfcol(11)
        s_wdown = s_packbf[:, 12 * C:12 * C + 64]   # W_down^T : [C, 64]
        s_mask = s_packf[:, 0:C]
        s_negb = s_packf[:, C:2 * C]
        s_iotar = s_packf[:, 2 * C:3 * C]
        s_rankrow = s_packf[:, 3 * C:4 * C]
        s_bgate = s_packf[:, 4 * C + 0:4 * C + 1]
        s_bpu = s_packf[:, 4 * C + 2:4 * C + 3]
        s_temp = s_packf[:, 4 * C + 3:4 * C + 4]
        s_rank = s_packf[:, 4 * C + 4:4 * C + 5]
        s_iotac = s_packf[:, 4 * C + 5:4 * C + 6]

        # ---- persistent state -------------------------------------------
        dw = [dwbig.tile([C, L], BF16, tag=f"dw{p}", name=f"dw{p}") for p in range(3)]
        dw8 = dwbig.tile([C, 2, L], FP8, tag="dw8", name="dw8")  # q,k scaled 2^5 for pass-2 pair
        gram_acc = stat.tile([C, C], F32, tag="gram", name="gram")
        sqsums = stat.tile([C, 2, NCH], F32, tag="sqsums", name="sqsums")  # q/k row-sumsq partials
        spack = stat.tile([C, 16], F32, tag="spack", name="spack")

        # double-buffered gap-layout x chunks (bf16 for v, fp8 for q/k): the
        # interior [2:] is fully DMA-overwritten every chunk; only the 2-col
        # lead gap needs zeroing, once
        xbb_bufs, xb8_bufs = [], []
        for i in range(2):
            xbb_bufs.append(stat.tile([C, XBCOLS], BF16, tag=f"xbb{i}", name=f"xbb{i}"))
            xb8_bufs.append(stat.tile([C, XBCOLS], FP8, tag=f"xb8{i}", name=f"xb8{i}"))
        for i in range(2):
            nc.vector.memset(xbb_bufs[i][:, XBCOLS - 2:XBCOLS], 0.0)
            nc.vector.memset(xb8_bufs[i][:, XBCOLS - 2:XBCOLS], 0.0)
        nc.vector.memset(gram_acc[:, :], 0.0)

        NSL = CHUNK_ROWS // 4
        with ExitStack() as p1:
            trp = p1.enter_context(tc.tile_pool(name="trp", bufs=2))
            scrp = p1.enter_context(tc.tile_pool(name="scrp", bufs=2))
            psdw = p1.enter_context(tc.tile_pool(name="psdw", bufs=6, space="PSUM"))
            psg = p1.enter_context(tc.tile_pool(name="psg", bufs=2, space="PSUM"))

            def issue_x_prefetch(ch):
                r0 = ch * CHUNK_ROWS
                nc.sync.dma_start(
                    out=xbb_bufs[ch % 2][:, 0:ROWSPAN * GAPW],
                    in_=xbf_h[:, r0 * GAPW:(r0 + ROWSPAN) * GAPW])
                nc.gpsimd.dma_start(
                    out=xb8_bufs[ch % 2][:, 0:ROWSPAN * GAPW],
                    in_=x8_h[:, r0 * GAPW:(r0 + ROWSPAN) * GAPW])

            def issue_gram(trq, trk):
                # Gram partial: G0 += q_ch @ k_ch^T (runs a chunk behind so
                # the PE never waits on the transposes)
                gps = psg.tile([C, C], F32, tag="gps", name="gps")
                for j in range(CHUNK_ROWS):
                    nc.tensor.matmul(gps[:, :], trq[:, j, :], trk[:, j, :],
                                     start=(j == 0), stop=(j == CHUNK_ROWS - 1))
                nc.vector.tensor_add(gram_acc[:, :], gram_acc[:, :], gps[:, :])

            # chunk-0 x before the (late-needed) constant DMAs for fast start;
            # the small constants ride the scalar queue so gpsimd only carries
            # the x8 chunk stream
            issue_x_prefetch(0)
            nc.scalar.dma_start(out=s_packf[:, :], in_=packf_h[:, :])
            nc.scalar.dma_start(out=s_rs[:, :], in_=rs_h[:, :])
            nc.scalar.dma_start(out=s_wpu[:, :], in_=wpu_h[:, :])

            # hoist the permutation-matrix build (depends only on constants)
            # so the small phase after the chunk loop is shorter. P and P^T
            # are built directly on DVE from rank/iota compares - no PE
            # transpose needed.
            ptf = stat.tile([C, C], F32, tag="ptf", name="ptf")
            nc.vector.tensor_scalar(out=ptf[:, :], in0=s_iotar[:, :], scalar1=s_rank[:, :],
                                    scalar2=None, op0=mybir.AluOpType.is_equal)
            pt_bf = stat.tile([C, C], BF16, tag="pt_bf", name="pt_bf")
            nc.vector.tensor_copy(out=pt_bf[:, :], in_=ptf[:, :])
            p_bf = stat.tile([C, C], BF16, tag="p_bf", name="p_bf")
            nc.vector.tensor_scalar(out=p_bf[:, :], in0=s_rankrow[:, :],
                                    scalar1=s_iotac[:, :], scalar2=None,
                                    op0=mybir.AluOpType.is_equal)

            prev_tr = None
            for ch in range(NCH):
                r0 = ch * CHUNK_ROWS
                if ch + 1 < NCH:
                    issue_x_prefetch(ch + 1)
                xbb = xbb_bufs[ch % 2]
                xb8 = xb8_bufs[ch % 2]

                tr_tiles = {}
                # ---- v: 9 folded bf16 taps (accuracy-critical plane)
                for sl in range(NSL):
                    j0 = sl * 4
                    pd = psdw.tile([C, 4 * W], F32, tag="psdw", name="psdw")
                    for i, (dy, dx) in enumerate(PE_TAPS):
                        rhs = view3(xbb, 2 + (1 + j0 + dy) * GAPW + dx, 4, GAPW, W)
                        nc.tensor.matmul(pd[:, :], s_wv_taps[i][:, :], rhs,
                                         start=(i == 0), stop=(i == 8))
                    nc.scalar.activation(
                        out=dw[2][:, (r0 + j0) * W:(r0 + j0 + 4) * W], in_=pd[:, :],
                        func=AF.Copy)

                for p in (0, 1):
                    dwbuf = dw[p]
                    for sl in range(NSL):
                        j0 = sl * 4
                        pd = psdw.tile([C, 4 * W], F32, tag="psdw", name="psdw")
                        # q/k: 5 DoubleRow pairs (9 taps + 1 zero pad)
                        for i in range(5):
                            ta = PE_TAPS[2 * i]
                            if i < 4:
                                tb = PE_TAPS[2 * i + 1]
                                d = (tb[0] - ta[0]) * GAPW + (tb[1] - ta[1])
                            else:
                                d = 1  # zero-weighted pad tap
                            lhsT = view3(s_pack8, (p * 10 + 2 * i) * C, 2, C, C)
                            rhs = view4(xb8, 2 + (1 + j0 + ta[0]) * GAPW + ta[1],
                                        d, 4, GAPW, W)
                            nc.tensor.matmul(pd[:, :], lhsT, rhs,
                                             start=(i == 0), stop=(i == 4),
                                             perf_mode=DR)
                        nc.scalar.activation(
                            out=dwbuf[:, (r0 + j0) * W:(r0 + j0 + 4) * W], in_=pd[:, :],
                            func=AF.Copy, scale=1.0 / W8SCALE)

                    # ---- fp8 copy, row sum-of-squares, transpose for q,k
                    chsl = dwbuf[:, r0 * W:(r0 + CHUNK_ROWS) * W]
                    nc.vector.tensor_scalar_mul(
                        out=dw8[:, p, r0 * W:(r0 + CHUNK_ROWS) * W], in0=chsl,
                        scalar1=2.0 ** 5)
                    scr = scrp.tile([C, CHUNK_ROWS * W], BF16, tag="sqscr", name="sqscr")
                    nc.vector.scalar_tensor_tensor(
                        out=scr[:, :], in0=chsl, scalar=0.0, in1=chsl,
                        op0=mybir.AluOpType.bypass, op1=MULT,
                        accum_out=sqsums[:, p, ch:ch + 1])
                    tr = trp.tile([C, CHUNK_ROWS, W], BF16, tag=f"tr{p}", name=f"tr{p}")
                    nc.sync.dma_start_transpose(out=tr[:, :, :], in_=chsl)
                    tr_tiles[p] = tr

                # lagged by one chunk: gram (PE) for ch-1 so the PE never
                # waits on the transposes
                if ch > 0:
                    issue_gram(prev_tr[0], prev_tr[1])
                prev_tr = tr_tiles

            issue_gram(prev_tr[0], prev_tr[1])

        # ================= small-matrix phase ============================
        with ExitStack() as sm:
            smp = sm.enter_context(tc.tile_pool(name="smp", bufs=1))
            pss_ctx = ExitStack()
            pss = pss_ctx.enter_context(tc.tile_pool(name="pss", bufs=2, space="PSUM"))

            # rnorm_q / rnorm_k; PDqT / PDkT
            pd8 = smp.tile([C, 2 * C], FP8, tag="pd8", name="pd8")
            pd_bf = []
            for pi in range(2):
                nc.vector.tensor_reduce(out=spack[:, 3 + pi:4 + pi], in_=sqsums[:, pi, :],
                                        axis=mybir.AxisListType.X, op=ADD)
                nc.scalar.activation(out=spack[:, 5 + pi:6 + pi], in_=spack[:, 3 + pi:4 + pi],
                                     func=AF.Sqrt)
                nc.vector.tensor_scalar_max(out=spack[:, 5 + pi:6 + pi],
                                            in0=spack[:, 5 + pi:6 + pi], scalar1=1e-12)
                nc.vector.reciprocal(out=spack[:, 5 + pi:6 + pi], in_=spack[:, 5 + pi:6 + pi])
                t = smp.tile([C, C], BF16, tag=f"pd{pi}", name=f"pd{pi}")
                nc.vector.tensor_scalar_mul(out=t[:, :], in0=ptf[:, :],
                                            scalar1=spack[:, 5 + pi:6 + pi])
                pd_bf.append(t)
                # fp8 copy (x 2^-5 to undo the dw8 scale) for the pass-2 pair
                nc.vector.tensor_scalar_mul(out=spack[:, 13:14],
                                            in0=spack[:, 5 + pi:6 + pi],
                                            scalar1=2.0 ** -5)
                nc.vector.tensor_scalar_mul(out=pd8[:, pi * C:(pi + 1) * C],
                                            in0=ptf[:, :], scalar1=spack[:, 13:14])

            # v0 = (qsum*rnq + ksum*rnk)/L  (qsum/ksum exact from host)
            nc.vector.tensor_mul(spack[:, 8:9], s_rs[:, 0:1], spack[:, 5:6])
            nc.vector.tensor_mul(spack[:, 9:10], s_rs[:, 1:2], spack[:, 6:7])
            nc.vector.tensor_add(spack[:, 8:9], spack[:, 8:9], spack[:, 9:10])
            nc.vector.tensor_scalar_mul(out=spack[:, 8:9], in0=spack[:, 8:9], scalar1=1.0 / L)
            sout = smp.tile([C, 4], F32, tag="sout", name="sout")
            nc.vector.tensor_copy(out=sout[:, 0:1], in_=spack[:, 8:9])   # v0
            nc.vector.tensor_copy(out=sout[:, 1:2], in_=s_rank[:, :])    # rank echo
            nc.vector.tensor_copy(out=sout[:, 2:3], in_=s_rs[:, 0:1])    # qsum echo
            nc.vector.tensor_copy(out=sout[:, 3:4], in_=spack[:, 3:4])   # ssq_q
            nc.sync.dma_start(out=stats_h[:, :], in_=sout[:, :])

            # Gp = (P Dq) G0 (Dk P^T)
            g0_bf = smp.tile([C, C], BF16, tag="g0bf", name="g0bf")
            nc.vector.tensor_copy(out=g0_bf[:, :], in_=gram_acc[:, :])
            t1ps = pss.tile([C, C], F32, tag="psf", name="psf")
            nc.tensor.matmul(t1ps[:, :], pd_bf[0][:, :], g0_bf[:, :], start=True, stop=True)
            t1_bf = smp.tile([C, C], BF16, tag="t1bf", name="t1bf")
            nc.scalar.copy(out=t1_bf[:, :], in_=t1ps[:, :])
            t1tps = pss.tile([C, C], BF16, tag="psbf", name="psbf")
            nc.tensor.transpose(t1tps[:, :], t1_bf[:, :], s_ident[:, :])
            t1t_bf = smp.tile([C, C], BF16, tag="t1tbf", name="t1tbf")
            nc.scalar.copy(out=t1t_bf[:, :], in_=t1tps[:, :])
            gpps = pss.tile([C, C], F32, tag="psf", name="psf")
            nc.tensor.matmul(gpps[:, :], t1t_bf[:, :], pd_bf[1][:, :], start=True, stop=True)

            # masked block-diagonal softmax (rows = rank space)
            xsm = smp.tile([C, C], F32, tag="xsm", name="xsm")
            nc.vector.tensor_scalar_mul(out=xsm[:, :], in0=gpps[:, :], scalar1=s_temp[:, :])
            nc.vector.tensor_mul(xsm[:, :], xsm[:, :], s_mask[:, :])
            nc.vector.tensor_add(xsm[:, :], xsm[:, :], s_negb[:, :])
            nc.vector.tensor_reduce(out=spack[:, 11:12], in_=xsm[:, :],
                                    axis=mybir.AxisListType.X, op=mybir.AluOpType.max)
            nc.vector.tensor_scalar_sub(out=xsm[:, :], in0=xsm[:, :], scalar1=spack[:, 11:12])
            nc.scalar.activation(out=xsm[:, :], in_=xsm[:, :], func=AF.Exp,
                                 accum_out=spack[:, 12:13])
            nc.vector.reciprocal(out=spack[:, 12:13], in_=spack[:, 12:13])
            a_bf = smp.tile([C, C], BF16, tag="a_bf", name="a_bf")
            nc.vector.tensor_scalar_mul(out=a_bf[:, :], in0=xsm[:, :], scalar1=spack[:, 12:13])

            # W_v = P^T A^T  (lhsT for the attention matmul over v)
            atps = pss.tile([C, C], BF16, tag="psbf", name="psbf")
            nc.tensor.transpose(atps[:, :], a_bf[:, :], s_ident[:, :])
            at_bf = smp.tile([C, C], BF16, tag="at_bf", name="at_bf")
            nc.scalar.copy(out=at_bf[:, :], in_=atps[:, :])
            wvps = pss.tile([C, C], F32, tag="psf", name="psf")
            nc.tensor.matmul(wvps[:, :], p_bf[:, :], at_bf[:, :], start=True, stop=True)
            wv_bf = smp.tile([C, C], BF16, tag="wv_bf", name="wv_bf")
            nc.scalar.copy(out=wv_bf[:, :], in_=wvps[:, :])

            # wpa = (W_proj A P)^T = P^T A^T W_proj^T
            m1ps = pss.tile([C, C], F32, tag="psf", name="psf2")
            nc.tensor.matmul(m1ps[:, :], a_bf[:, :], s_wproj[:, :], start=True, stop=True)
            m1_bf = smp.tile([C, C], BF16, tag="m1_bf", name="m1_bf")
            nc.scalar.copy(out=m1_bf[:, :], in_=m1ps[:, :])
            wpaps = pss.tile([C, C], F32, tag="psf", name="psf3")
            nc.tensor.matmul(wpaps[:, :], p_bf[:, :], m1_bf[:, :], start=True, stop=True)
            wpa_bf = smp.tile([C, C], BF16, tag="wpa_bf", name="wpa_bf")
            nc.scalar.copy(out=wpa_bf[:, :], in_=wpaps[:, :])

            pss_ctx.close()

            # ============== pass 2: streamed output ======================
            with ExitStack() as p2x:
                p2 = p2x.enter_context(tc.tile_pool(name="p2", bufs=3))
                psS = p2x.enter_context(tc.tile_pool(name="psS", bufs=2, space="PSUM"))
                psG2 = p2x.enter_context(tc.tile_pool(name="psG2", bufs=2, space="PSUM"))
                psD = p2x.enter_context(tc.tile_pool(name="psD", bufs=1, space="PSUM"))
                psO = p2x.enter_context(tc.tile_pool(name="psO", bufs=2, space="PSUM"))

                for t in range(N_P2):
                    c0 = t * P2_TILE
                    q_sl = dw[0][:, c0:c0 + P2_TILE]
                    k_sl = dw[1][:, c0:c0 + P2_TILE]
                    v_sl = dw[2][:, c0:c0 + P2_TILE]

                    # sxy = (A P) v + (P Dq) q + (P Dk) k; the q,k terms run
                    # as one fp8 DoubleRow pair over the dw8 copies
                    ps = psS.tile([C, P2_TILE], F32, tag="ps", name="ps")
                    nc.tensor.matmul(ps[:, :], wv_bf[:, :], v_sl, start=True, stop=False)
                    nc.tensor.matmul(ps[:, :], view3(pd8, 0, 2, C, C),
                                     view3(dw8, c0, 2, L, P2_TILE),
                                     start=False, stop=True, perf_mode=DR)
                    sxy_sb = p2.tile([C, P2_TILE], BF16, tag="sxy", name="sxy_sb")
                    nc.vector.tensor_copy(out=sxy_sb[:, :], in_=ps[:, :])

                    pg = psG2.tile([C, P2_TILE], F32, tag="pg", name="pg")
                    nc.tensor.matmul(pg[:, :], s_wgate[:, :], sxy_sb[:, :], start=True, stop=True)
                    gat_sb = p2.tile([C, P2_TILE], BF16, tag="gat", name="gat_sb")
                    nc.scalar.activation(out=gat_sb[:, :], in_=pg[:, :], func=AF.Gelu,
                                         bias=s_bgate[:, :], scale=1.0)
                    gated_sb = p2.tile([C, P2_TILE], BF16, tag="gated", name="gated_sb")
                    nc.vector.tensor_mul(gated_sb[:, :], gat_sb[:, :], sxy_sb[:, :])

                    pdn = psD.tile([64, P2_TILE], F32, tag="pd2", name="pdn")
                    nc.tensor.matmul(pdn[:, :], s_wdown[:, :], gated_sb[:, :],
                                     start=True, stop=True)
                    d_sb = p2.tile([64, P2_TILE], BF16, tag="dsb", name="d_sb")
                    nc.scalar.activation(out=d_sb[:, :], in_=pdn[:, :], func=AF.Identity,
                                         bias=s_bdown[:, :], scale=1.0)

                    # out = (W_proj A P) v + (W_proj W_up) d  (+ W_proj b_up bias)
                    po = psO.tile([C, P2_TILE], F32, tag="po", name="po")
                    nc.tensor.matmul(po[:, :], wpa_bf[:, :], v_sl, start=True, stop=False)
                    nc.tensor.matmul(po[:, :], s_wpu[:, :], d_sb[:, :],
                                     start=False, stop=True)
                    outf = p2.tile([C, P2_TILE], F32, tag="outf", name="outf")
                    nc.scalar.activation(out=outf[:, :], in_=po[:, :], func=AF.Identity,
                                         bias=s_bpu[:, :], scale=1.0)
                    nc.sync.dma_start(out=out1_h[:, c0:c0 + P2_TILE], in_=outf[:, :])


_NC_CACHE = None


def _get_nc():
    global _NC_CACHE
    if _NC_CACHE is None:
        _NC_CACHE = build_bass()
    return _NC_CACHE


def _host_inputs(x, temperature, w_qkv, w_dw, w_proj, w_gate, b_gate,
                 w_down, b_down, w_up, b_up):
    f = np.float32
    x = np.asarray(x, f).reshape(B, C, L)
    w_qkv = np.asarray(w_qkv, f)
    w_dw = np.asarray(w_dw, f)
    temperature = np.asarray(temperature, f)

    shared = {}
    packbf = np.zeros((C, 12 * C + 64), np.float32)
    for ti, (dy, dx) in enumerate(PE_TAPS):
        wt = w_dw[2 * C:3 * C, 0, dy + 1, dx + 1]
        packbf[:, ti * C:(ti + 1) * C] = (w_qkv[2 * C:3 * C, :] * wt[:, None]).T
    packbf[:, 9 * C:10 * C] = np.asarray(w_gate, f).T
    packbf[:, 10 * C:11 * C] = np.asarray(w_proj, f).T
    packbf[:, 11 * C:12 * C] = np.eye(C, dtype=f)
    packbf[:, 12 * C:12 * C + 64] = np.asarray(w_down, f).T
    shared["packbf"] = packbf.astype(BD)
    wpu = np.asarray(w_proj, f) @ np.asarray(w_up, f)          # (C, 64)
    shared["wpu_t"] = np.ascontiguousarray(wpu.T).astype(BD)   # (64, C)

    pack8 = np.zeros((C, 20 * C), np.float32)
    for p in range(2):
        for ti, (dy, dx) in enumerate(PE_TAPS):
            wt = w_dw[p * C:(p + 1) * C, 0, dy + 1, dx + 1]
            pack8[:, (p * 10 + ti) * C:(p * 10 + ti + 1) * C] = \
                (w_qkv[p * C:(p + 1) * C, :] * wt[:, None]).T * W8SCALE
    shared["pack8"] = pack8.astype(F8)

    # exact channel means of dwconv(Wq x) via rectangle sums (linear in x),
    # and exact per-image row sums for q and k planes
    xr = x.reshape(B, C, H, W).astype(np.float64)
    wq = w_qkv[:C, :].astype(np.float64)
    wk = w_qkv[C:2 * C, :].astype(np.float64)
    wdw_q = w_dw[:C, 0].astype(np.float64)
    wdw_k = w_dw[C:2 * C, 0].astype(np.float64)
    mean = np.zeros(C, np.float64)
    rs_q = np.zeros((B, C), np.float64)
    rs_k = np.zeros((B, C), np.float64)
    for dy in (-1, 0, 1):
        for dx in (-1, 0, 1):
            y0, y1 = max(0, dy), min(H - 1, H - 1 + dy)
            x0, x1 = max(0, dx), min(W - 1, W - 1 + dx)
            rect_b = xr[:, :, y0:y1 + 1, x0:x1 + 1].sum(axis=(2, 3))   # (B, C)
            rect = rect_b.sum(axis=0)
            mean += wdw_q[:, dy + 1, dx + 1] * (wq @ rect)
            rs_q += wdw_q[None, :, dy + 1, dx + 1] * (rect_b @ wq.T)
            rs_k += wdw_k[None, :, dy + 1, dx + 1] * (rect_b @ wk.T)
    mean /= float(B * L)
    idx_order = np.argsort(-mean, kind="stable")
    rank = np.empty(C, np.int64)
    rank[idx_order] = np.arange(C)

    gid = np.zeros(C, np.int64)
    s = 0
    for gi, g in enumerate(GROUP_SIZES):
        gid[s:s + g] = gi
        s += g
    same = (gid[:, None] == gid[None, :])
    idx = np.arange(C)
    packf = np.zeros((C, 4 * C + 15), f)
    packf[:, 0:C] = same.astype(f)
    packf[:, C:2 * C] = np.where(same, 0.0, -30000.0)
    packf[:, 2 * C:3 * C] = np.tile(idx[None, :], (C, 1))
    packf[:, 3 * C:4 * C] = np.tile(rank[None, :], (C, 1)).astype(f)
    packf[:, 4 * C + 0] = np.asarray(b_gate, f)
    packf[:, 4 * C + 1] = np.asarray(b_up, f)
    packf[:, 4 * C + 2] = np.asarray(w_proj, f) @ np.asarray(b_up, f)
    packf[:, 4 * C + 3] = temperature[gid, 0, 0]
    packf[:, 4 * C + 4] = rank.astype(f)
    packf[:, 4 * C + 5] = idx.astype(f)
    shared["packf"] = packf
    shared["bdown"] = np.asarray(b_down, f).reshape(64, 1)

    xg = np.zeros((B, C, H + 2, GAPW), np.float32)
    xg[:, :, 1:H + 1, 2:] = x.reshape(B, C, H, W)
    xg = xg.reshape(B, C, (H + 2) * GAPW)
    in_maps = []
    for i in range(NCORES):
        m = dict(shared)
        m["xbf"] = xg[i].astype(BD)
        m["x8"] = xg[i].astype(F8)
        m["rs"] = np.stack([rs_q[i], rs_k[i]], axis=1).astype(f)
        in_maps.append(m)
    return in_maps


def _assemble(results):
    out = np.zeros((B, C, H, W), np.float32)
    cache = np.zeros((B, C, H, W), np.float32)
    for i in range(NCORES):
        out[i] = np.asarray(results[i]["out1"], np.float32).reshape(C, H, W)
        st = np.asarray(results[i]["stats"], np.float32)
        v0 = st[:, 0]
        rank = np.rint(st[:, 1]).astype(np.int64)
        idx_dev = np.argsort(rank)
        mt = v0[idx_dev]
        s = 0
        gms = []
        for g in GROUP_SIZES:
            gm = mt[s:s + g]
            s += g
            rep = max(1, C // g)
            gm = np.tile(gm, rep)
            if gm.shape[0] >= C:
                gm = gm[:C]
            else:
                gm = np.pad(gm, (0, C - gm.shape[0]))
            gms.append(gm)
        acc = np.mean(np.stack(gms, 0), 0)
        cache[i] = np.broadcast_to((acc * 0.9)[:, None, None], (C, H, W))
    return out, cache


def kernel(**inputs):
    nc = _get_nc()
    in_maps = _host_inputs(**inputs)
    res = run_bass_kernel_spmd(nc, in_maps, list(range(NCORES)))
    return _assemble(res.results)


if __name__ == "__main__":
    rng = np.random.default_rng(0)
    dummy = {
        "x": rng.standard_normal((B, C, H, W), dtype=np.float32),
        "temperature": np.ones((4, 1, 1), np.float32),
        "w_qkv": rng.standard_normal((3 * C, C), dtype=np.float32) * 0.02,
        "w_dw": rng.standard_normal((3 * C, 1, 3, 3), dtype=np.float32) * 0.02,
        "w_proj": rng.standard_normal((C, C), dtype=np.float32) * 0.02,
        "w_gate": rng.standard_normal((C, C), dtype=np.float32) * 0.02,
        "b_gate": np.zeros(C, np.float32),
        "b_down": np.zeros(C // 2, np.float32),
        "w_down": rng.standard_normal((C // 2, C), dtype=np.float32) * 0.02,
        "w_up": rng.standard_normal((C, C // 2), dtype=np.float32) * 0.02,
        "b_up": np.zeros(C, np.float32),
    }
    o, c = kernel(**dummy)
    print("out", o.shape, o.dtype, "cache", c.shape, c.dtype)


# revision 30
# speedup vs baseline: 1.1954x; 1.0084x over previous
"""Trainium2 Bass kernel for nn_Attention_34806414967022 (sparse channel attention).

Data-parallel over batch: 8 batch images -> 8 NeuronCores, one image each.

Per-core pipeline (image = (C=128, H=128, W=128), L = H*W = 16384):
  1. Depthwise-3x3(Conv1x1(x)) planes, fully folded into PE matmuls
     (W_eff[tap] = diag(w_dw[:,tap]) @ W_qkv applied to shifted x views):
       - q,k planes: fp8(e4m3) weights+activations, taps fused in pairs via
         MatmulPerfMode.DoubleRow (2 taps per instruction at 2x rate).
         Weights pre-scaled by 2^13 on host, descaled in the PSUM evacuation.
       - v plane: bf16 (accuracy-critical: attn@v feeds the output directly).
     Shift safety: x chunks live in a gap layout (row stride 130, two zero gap
     cols) so +-1 column shifts read zeros at row edges. x arrives pre-cast
     from the host (bf16 + fp8 copies) and is DMAed straight into the gap
     layout - no on-device cast.
  2. Channel stats: row sums of q,k are computed EXACTLY on the host via
     rectangle sums (linear in x) and passed in. On-device: row sums of
     squares (DVE STT bypass*self trick), Gram G0 = q @ k^T via DMA-transposed
     tiles accumulated on PE.
  3. Permutation matrix P from host-provided ranks; attention computed as
     dense 128x128 with block-diagonal masking.
  4. out = W_proj(mod + attn) streamed over L; the up-projection is folded
     into W_proj on the host (W_pu = W_proj @ W_up) saving a matmul per tile.

Outputs per core: out1 (C,L) fp32 and stats (C,4) fp32 [v0, rank, -, ssq_q].
Host assembles the qv_cache output (broadcast of a length-128 vector) in numpy.
"""

import sys

sys.path.insert(0, "/opt/trn_rl_repo")

import numpy as np
import ml_dtypes
from contextlib import ExitStack

import concourse.bass as bass
import concourse.bacc as bacc
import concourse.tile as tile

from concourse import mybir
from concourse.bass_utils import run_bass_kernel_spmd

F32 = mybir.dt.float32
BF16 = mybir.dt.bfloat16
FP8 = mybir.dt.float8e4
BD = ml_dtypes.bfloat16
F8 = ml_dtypes.float8_e4m3

C = 128
H = 128
W = 128
L = H * W
B = 8
NCORES = 8
GROUP_SIZES = [16, 32, 32, 48]

CHUNK_ROWS = 16
NCH = H // CHUNK_ROWS
ROWSPAN = CHUNK_ROWS + 2          # chunk rows + 1 halo row each side
GAPW = W + 2                      # image row + 2 zero gap cols
XBCOLS = 2 + ROWSPAN * GAPW      # leading 2-col zero gap keeps rows 4B aligned

PE_TAPS = [(dy, dx) for dy in (-1, 0, 1) for dx in (-1, 0, 1)]
W8SCALE = 2.0 ** 13               # fp8 weight pre-scale (descaled on PSUM copy)
VPROWS = H + 2                    # full-plane v_pre gap buffer (1 pad row each side)
VPCOLS = 2 + VPROWS * GAPW

P2_TILE = 512
N_P2 = L // P2_TILE

ADD = mybir.AluOpType.add
MULT = mybir.AluOpType.mult
AF = mybir.ActivationFunctionType
DR = mybir.MatmulPerfMode.DoubleRow


def view3(t, off, rows, rowstride, w):
    """Strided 3D view into a 2D sbuf tile: (partitions, rows, w)."""
    return bass.AP(tensor=t.tensor, offset=t.offset + off, ap=[t.ap[0], [rowstride, rows], [1, w]])


def view4(t, off, dpair, rows, rowstride, w):
    """(partitions, 2, rows, w) pair view for DoubleRow matmuls."""
    return bass.AP(tensor=t.tensor, offset=t.offset + off,
                   ap=[t.ap[0], [dpair, 2], [rowstride, rows], [1, w]])


def build_bass():
    nc = bacc.Bacc()
    _build_body(nc)
    nc.compile()
    return nc


def _build_body(nc):
    # ---- dram parameters -------------------------------------------------
    # x arrives from the host already in the padded gap layout (one zero pad
    # row top+bottom, 2 zero gap cols per row) so every chunk DMA is one
    # contiguous 2340B-per-partition descriptor instead of 18 strided 256B ones
    xbf_h = nc.declare_dram_parameter("xbf", [C, (H + 2) * GAPW], BF16, isOutput=False)
    x8_h = nc.declare_dram_parameter("x8", [C, (H + 2) * GAPW], FP8, isOutput=False)
    NBF = 12 * C + 64  # wv-taps*9 wgate wproj ident (C cols each) + wdown(64)
    NF32 = 4 * C + 15  # mask negb iotar rankrow + [bgate bup bpu temp rank iotac] + wdw_v*9
    packbf_h = nc.declare_dram_parameter("packbf", [C, NBF], BF16, isOutput=False)
    pack8_h = nc.declare_dram_parameter("pack8", [C, 20 * C], FP8, isOutput=False)
    wpu_h = nc.declare_dram_parameter("wpu_t", [64, C], BF16, isOutput=False)
    packf_h = nc.declare_dram_parameter("packf", [C, NF32], F32, isOutput=False)
    rs_h = nc.declare_dram_parameter("rs", [C, 2], F32, isOutput=False)
    bdown_h = nc.declare_dram_parameter("bdown", [64, 1], F32, isOutput=False)
    out1_h = nc.declare_dram_parameter("out1", [C, L], F32, isOutput=True)
    stats_h = nc.declare_dram_parameter("stats", [C, 4], F32, isOutput=True)

    with tile.TileContext(nc) as tc, ExitStack() as ctx:
        singles = ctx.enter_context(tc.tile_pool(name="singles", bufs=1))
        stat = ctx.enter_context(tc.tile_pool(name="stat", bufs=1))
        dwbig = ctx.enter_context(tc.tile_pool(name="dwbig", bufs=1))

        # ---- constants to SBUF: packed DMAs on separate queues ----------
        s_packbf = singles.tile([C, NBF], BF16, tag="s_packbf", name="s_packbf")
        nc.scalar.dma_start(out=s_packbf[:, :], in_=packbf_h[:, :])
        s_pack8 = singles.tile([C, 20 * C], FP8, tag="s_pack8", name="s_pack8")
        nc.scalar.dma_start(out=s_pack8[:, :], in_=pack8_h[:, :])
        s_packf = singles.tile([C, NF32], F32, tag="s_packf", name="s_packf")
        s_rs = singles.tile([C, 2], F32, tag="s_rs", name="s_rs")
        s_wpu = singles.tile([64, C], BF16, tag="s_wpu", name="s_wpu")
        s_bdown = singles.tile([64, 1], F32, tag="s_bdown", name="s_bdown")
        nc.scalar.dma_start(out=s_bdown[:, :], in_=bdown_h[:, :])

        def bfcol(i):
            return s_packbf[:, i * C:(i + 1) * C]

        s_wv_taps = [bfcol(i) for i in range(9)]
        s_wgate = bfcol(9)
        s_wproj = bfcol(10)
        s_ident = bfcol(11)
        s_wdown = s_packbf[:, 12 * C:12 * C + 64]   # W_down^T : [C, 64]
        s_mask = s_packf[:, 0:C]
        s_negb = s_packf[:, C:2 * C]
        s_iotar = s_packf[:, 2 * C:3 * C]
        s_rankrow = s_packf[:, 3 * C:4 * C]
        s_bgate = s_packf[:, 4 * C + 0:4 * C + 1]
        s_bpu = s_packf[:, 4 * C + 2:4 * C + 3]
        s_temp = s_packf[:, 4 * C + 3:4 * C + 4]
        s_rank = s_packf[:, 4 * C + 4:4 * C + 5]
        s_iotac = s_packf[:, 4 * C + 5:4 * C + 6]

        # ---- persistent state -------------------------------------------
        dw = [dwbig.tile([C, L], BF16, tag=f"dw{p}", name=f"dw{p}") for p in range(3)]
        dw8 = dwbig.tile([C, 2, L], FP8, tag="dw8", name="dw8")  # q,k scaled 2^5 for pass-2 pair
        gram_acc = stat.tile([C, C], F32, tag="gram", name="gram")
        sqsums = stat.tile([C, 2, NCH], F32, tag="sqsums", name="sqsums")  # q/k row-sumsq partials
        spack = stat.tile([C, 16], F32, tag="spack", name="spack")

        # double-buffered gap-layout x chunks (bf16 for v, fp8 for q/k): the
        # interior [2:] is fully DMA-overwritten every chunk; only the 2-col
        # lead gap needs zeroing, once
        xbb_bufs, xb8_bufs = [], []
        for i in range(2):
            xbb_bufs.append(stat.tile([C, XBCOLS], BF16, tag=f"xbb{i}", name=f"xbb{i}"))
            xb8_bufs.append(stat.tile([C, XBCOLS], FP8, tag=f"xb8{i}", name=f"xb8{i}"))
        for i in range(2):
            nc.vector.memset(xbb_bufs[i][:, XBCOLS - 2:XBCOLS], 0.0)
            nc.vector.memset(xb8_bufs[i][:, XBCOLS - 2:XBCOLS], 0.0)
        nc.vector.memset(gram_acc[:, :], 0.0)

        NSL = CHUNK_ROWS // 4
        with ExitStack() as p1:
            trp = p1.enter_context(tc.tile_pool(name="trp", bufs=2))
            scrp = p1.enter_context(tc.tile_pool(name="scrp", bufs=2))
            psdw = p1.enter_context(tc.tile_pool(name="psdw", bufs=6, space="PSUM"))
            psg = p1.enter_context(tc.tile_pool(name="psg", bufs=2, space="PSUM"))

            def issue_x_prefetch(ch):
                r0 = ch * CHUNK_ROWS
                nc.sync.dma_start(
                    out=xbb_bufs[ch % 2][:, 0:ROWSPAN * GAPW],
                    in_=xbf_h[:, r0 * GAPW:(r0 + ROWSPAN) * GAPW])
                nc.gpsimd.dma_start(
                    out=xb8_bufs[ch % 2][:, 0:ROWSPAN * GAPW],
                    in_=x8_h[:, r0 * GAPW:(r0 + ROWSPAN) * GAPW])

            def issue_gram(trq, trk):
                # Gram partial: G0 += q_ch @ k_ch^T (runs a chunk behind so
                # the PE never waits on the transposes)
                gps = psg.tile([C, C], F32, tag="gps", name="gps")
                for j in range(CHUNK_ROWS):
                    nc.tensor.matmul(gps[:, :], trq[:, j, :], trk[:, j, :],
                                     start=(j == 0), stop=(j == CHUNK_ROWS - 1))
                nc.vector.tensor_add(gram_acc[:, :], gram_acc[:, :], gps[:, :])

            # chunk-0 x before the (late-needed) constant DMAs for fast start;
            # the small constants ride the scalar queue so gpsimd only carries
            # the x8 chunk stream
            issue_x_prefetch(0)
            nc.scalar.dma_start(out=s_packf[:, :], in_=packf_h[:, :])
            nc.scalar.dma_start(out=s_rs[:, :], in_=rs_h[:, :])
            nc.scalar.dma_start(out=s_wpu[:, :], in_=wpu_h[:, :])

            # hoist the permutation-matrix build (depends only on constants)
            # so the small phase after the chunk loop is shorter. P and P^T
            # are built directly on DVE from rank/iota compares - no PE
            # transpose needed.
            ptf = stat.tile([C, C], F32, tag="ptf", name="ptf")
            nc.vector.tensor_scalar(out=ptf[:, :], in0=s_iotar[:, :], scalar1=s_rank[:, :],
                                    scalar2=None, op0=mybir.AluOpType.is_equal)
            pt_bf = stat.tile([C, C], BF16, tag="pt_bf", name="pt_bf")
            nc.vector.tensor_copy(out=pt_bf[:, :], in_=ptf[:, :])
            p_bf = stat.tile([C, C], BF16, tag="p_bf", name="p_bf")
            nc.vector.tensor_scalar(out=p_bf[:, :], in0=s_rankrow[:, :],
                                    scalar1=s_iotac[:, :], scalar2=None,
                                    op0=mybir.AluOpType.is_equal)

            prev_tr = None
            for ch in range(NCH):
                r0 = ch * CHUNK_ROWS
                if ch + 1 < NCH:
                    issue_x_prefetch(ch + 1)
                xbb = xbb_bufs[ch % 2]
                xb8 = xb8_bufs[ch % 2]

                tr_tiles = {}
                # ---- v: 9 folded bf16 taps (accuracy-critical plane)
                for sl in range(NSL):
                    j0 = sl * 4
                    pd = psdw.tile([C, 4 * W], F32, tag="psdw", name="psdw")
                    for i, (dy, dx) in enumerate(PE_TAPS):
                        rhs = view3(xbb, 2 + (1 + j0 + dy) * GAPW + dx, 4, GAPW, W)
                        nc.tensor.matmul(pd[:, :], s_wv_taps[i][:, :], rhs,
                                         start=(i == 0), stop=(i == 8))
                    nc.scalar.activation(
                        out=dw[2][:, (r0 + j0) * W:(r0 + j0 + 4) * W], in_=pd[:, :],
                        func=AF.Copy)

                for p in (0, 1):
                    dwbuf = dw[p]
                    for sl in range(NSL):
                        j0 = sl * 4
                        pd = psdw.tile([C, 4 * W], F32, tag="psdw", name="psdw")
                        # q/k: 5 DoubleRow pairs (9 taps + 1 zero pad)
                        for i in range(5):
                            ta = PE_TAPS[2 * i]
                            if i < 4:
                                tb = PE_TAPS[2 * i + 1]
                                d = (tb[0] - ta[0]) * GAPW + (tb[1] - ta[1])
                            else:
                                d = 1  # zero-weighted pad tap
                            lhsT = view3(s_pack8, (p * 10 + 2 * i) * C, 2, C, C)
                            rhs = view4(xb8, 2 + (1 + j0 + ta[0]) * GAPW + ta[1],
                                        d, 4, GAPW, W)
                            nc.tensor.matmul(pd[:, :], lhsT, rhs,
                                             start=(i == 0), stop=(i == 4),
                                             perf_mode=DR)
                        nc.scalar.activation(
                            out=dwbuf[:, (r0 + j0) * W:(r0 + j0 + 4) * W], in_=pd[:, :],
                            func=AF.Copy, scale=1.0 / W8SCALE)

                    # ---- fp8 copy, row sum-of-squares, transpose for q,k
                    chsl = dwbuf[:, r0 * W:(r0 + CHUNK_ROWS) * W]
                    nc.vector.tensor_scalar_mul(
                        out=dw8[:, p, r0 * W:(r0 + CHUNK_ROWS) * W], in0=chsl,
                        scalar1=2.0 ** 5)
                    scr = scrp.tile([C, CHUNK_ROWS * W], BF16, tag="sqscr", name="sqscr")
                    nc.scalar.activation(out=scr[:, :], in_=chsl, func=AF.Square,
                                         accum_out=sqsums[:, p, ch:ch + 1])
                    tr = trp.tile([C, CHUNK_ROWS, W], BF16, tag=f"tr{p}", name=f"tr{p}")
                    nc.sync.dma_start_transpose(out=tr[:, :, :], in_=chsl)
                    tr_tiles[p] = tr

                # lagged by one chunk: gram (PE) for ch-1 so the PE never
                # waits on the transposes
                if ch > 0:
                    issue_gram(prev_tr[0], prev_tr[1])
                prev_tr = tr_tiles

            issue_gram(prev_tr[0], prev_tr[1])

        # ================= small-matrix phase ============================
        with ExitStack() as sm:
            smp = sm.enter_context(tc.tile_pool(name="smp", bufs=1))
            pss_ctx = ExitStack()
            pss = pss_ctx.enter_context(tc.tile_pool(name="pss", bufs=2, space="PSUM"))

            # rnorm_q / rnorm_k; PDqT / PDkT
            pd8 = smp.tile([C, 2 * C], FP8, tag="pd8", name="pd8")
            pd_bf = []
            for pi in range(2):
                nc.vector.tensor_reduce(out=spack[:, 3 + pi:4 + pi], in_=sqsums[:, pi, :],
                                        axis=mybir.AxisListType.X, op=ADD)
                nc.scalar.activation(out=spack[:, 5 + pi:6 + pi], in_=spack[:, 3 + pi:4 + pi],
                                     func=AF.Sqrt)
                nc.vector.tensor_scalar_max(out=spack[:, 5 + pi:6 + pi],
                                            in0=spack[:, 5 + pi:6 + pi], scalar1=1e-12)
                nc.vector.reciprocal(out=spack[:, 5 + pi:6 + pi], in_=spack[:, 5 + pi:6 + pi])
                t = smp.tile([C, C], BF16, tag=f"pd{pi}", name=f"pd{pi}")
                nc.vector.tensor_scalar_mul(out=t[:, :], in0=ptf[:, :],
                                            scalar1=spack[:, 5 + pi:6 + pi])
                pd_bf.append(t)
                # fp8 copy (x 2^-5 to undo the dw8 scale) for the pass-2 pair
                nc.vector.tensor_scalar_mul(out=spack[:, 13:14],
                                            in0=spack[:, 5 + pi:6 + pi],
                                            scalar1=2.0 ** -5)
                nc.vector.tensor_scalar_mul(out=pd8[:, pi * C:(pi + 1) * C],
                                            in0=ptf[:, :], scalar1=spack[:, 13:14])

            # v0 = (qsum*rnq + ksum*rnk)/L  (qsum/ksum exact from host)
            nc.vector.tensor_mul(spack[:, 8:9], s_rs[:, 0:1], spack[:, 5:6])
            nc.vector.tensor_mul(spack[:, 9:10], s_rs[:, 1:2], spack[:, 6:7])
            nc.vector.tensor_add(spack[:, 8:9], spack[:, 8:9], spack[:, 9:10])
            nc.vector.tensor_scalar_mul(out=spack[:, 8:9], in0=spack[:, 8:9], scalar1=1.0 / L)
            sout = smp.tile([C, 4], F32, tag="sout", name="sout")
            nc.vector.tensor_copy(out=sout[:, 0:1], in_=spack[:, 8:9])   # v0
            nc.vector.tensor_copy(out=sout[:, 1:2], in_=s_rank[:, :])    # rank echo
            nc.vector.tensor_copy(out=sout[:, 2:3], in_=s_rs[:, 0:1])    # qsum echo
            nc.vector.tensor_copy(out=sout[:, 3:4], in_=spack[:, 3:4])   # ssq_q
            nc.sync.dma_start(out=stats_h[:, :], in_=sout[:, :])

            # Gp = (P Dq) G0 (Dk P^T)
            g0_bf = smp.tile([C, C], BF16, tag="g0bf", name="g0bf")
            nc.vector.tensor_copy(out=g0_bf[:, :], in_=gram_acc[:, :])
            t1ps = pss.tile([C, C], F32, tag="psf", name="psf")
            nc.tensor.matmul(t1ps[:, :], pd_bf[0][:, :], g0_bf[:, :], start=True, stop=True)
            t1_bf = smp.tile([C, C], BF16, tag="t1bf", name="t1bf")
            nc.scalar.copy(out=t1_bf[:, :], in_=t1ps[:, :])
            t1tps = pss.tile([C, C], BF16, tag="psbf", name="psbf")
            nc.tensor.transpose(t1tps[:, :], t1_bf[:, :], s_ident[:, :])
            t1t_bf = smp.tile([C, C], BF16, tag="t1tbf", name="t1tbf")
            nc.scalar.copy(out=t1t_bf[:, :], in_=t1tps[:, :])
            gpps = pss.tile([C, C], F32, tag="psf", name="psf")
            nc.tensor.matmul(gpps[:, :], t1t_bf[:, :], pd_bf[1][:, :], start=True, stop=True)

            # masked block-diagonal softmax (rows = rank space)
            xsm = smp.tile([C, C], F32, tag="xsm", name="xsm")
            nc.vector.tensor_scalar_mul(out=xsm[:, :], in0=gpps[:, :], scalar1=s_temp[:, :])
            nc.vector.tensor_mul(xsm[:, :], xsm[:, :], s_mask[:, :])
            nc.vector.tensor_add(xsm[:, :], xsm[:, :], s_negb[:, :])
            nc.vector.tensor_reduce(out=spack[:, 11:12], in_=xsm[:, :],
                                    axis=mybir.AxisListType.X, op=mybir.AluOpType.max)
            nc.vector.tensor_scalar_sub(out=xsm[:, :], in0=xsm[:, :], scalar1=spack[:, 11:12])
            nc.scalar.activation(out=xsm[:, :], in_=xsm[:, :], func=AF.Exp,
                                 accum_out=spack[:, 12:13])
            nc.vector.reciprocal(out=spack[:, 12:13], in_=spack[:, 12:13])
            a_bf = smp.tile([C, C], BF16, tag="a_bf", name="a_bf")
            nc.vector.tensor_scalar_mul(out=a_bf[:, :], in0=xsm[:, :], scalar1=spack[:, 12:13])

            # W_v = P^T A^T  (lhsT for the attention matmul over v)
            atps = pss.tile([C, C], BF16, tag="psbf", name="psbf")
            nc.tensor.transpose(atps[:, :], a_bf[:, :], s_ident[:, :])
            at_bf = smp.tile([C, C], BF16, tag="at_bf", name="at_bf")
            nc.scalar.copy(out=at_bf[:, :], in_=atps[:, :])
            wvps = pss.tile([C, C], F32, tag="psf", name="psf")
            nc.tensor.matmul(wvps[:, :], p_bf[:, :], at_bf[:, :], start=True, stop=True)
            wv_bf = smp.tile([C, C], BF16, tag="wv_bf", name="wv_bf")
            nc.scalar.copy(out=wv_bf[:, :], in_=wvps[:, :])

            # wpa = (W_proj A P)^T = P^T A^T W_proj^T
            m1ps = pss.tile([C, C], F32, tag="psf", name="psf2")
            nc.tensor.matmul(m1ps[:, :], a_bf[:, :], s_wproj[:, :], start=True, stop=True)
            m1_bf = smp.tile([C, C], BF16, tag="m1_bf", name="m1_bf")
            nc.scalar.copy(out=m1_bf[:, :], in_=m1ps[:, :])
            wpaps = pss.tile([C, C], F32, tag="psf", name="psf3")
            nc.tensor.matmul(wpaps[:, :], p_bf[:, :], m1_bf[:, :], start=True, stop=True)
            wpa_bf = smp.tile([C, C], BF16, tag="wpa_bf", name="wpa_bf")
            nc.scalar.copy(out=wpa_bf[:, :], in_=wpaps[:, :])

            pss_ctx.close()

            # ============== pass 2: streamed output ======================
            with ExitStack() as p2x:
                p2 = p2x.enter_context(tc.tile_pool(name="p2", bufs=3))
                psS = p2x.enter_context(tc.tile_pool(name="psS", bufs=2, space="PSUM"))
                psG2 = p2x.enter_context(tc.tile_pool(name="psG2", bufs=2, space="PSUM"))
                psD = p2x.enter_context(tc.tile_pool(name="psD", bufs=1, space="PSUM"))
                psO = p2x.enter_context(tc.tile_pool(name="psO", bufs=2, space="PSUM"))

                for t in range(N_P2):
                    c0 = t * P2_TILE
                    q_sl = dw[0][:, c0:c0 + P2_TILE]
                    k_sl = dw[1][:, c0:c0 + P2_TILE]
                    v_sl = dw[2][:, c0:c0 + P2_TILE]

                    # sxy = (A P) v + (P Dq) q + (P Dk) k; the q,k terms run
                    # as one fp8 DoubleRow pair over the dw8 copies
                    ps = psS.tile([C, P2_TILE], F32, tag="ps", name="ps")
                    nc.tensor.matmul(ps[:, :], wv_bf[:, :], v_sl, start=True, stop=False)
                    nc.tensor.matmul(ps[:, :], view3(pd8, 0, 2, C, C),
                                     view3(dw8, c0, 2, L, P2_TILE),
                                     start=False, stop=True, perf_mode=DR)
                    sxy_sb = p2.tile([C, P2_TILE], BF16, tag="sxy", name="sxy_sb")
                    nc.vector.tensor_copy(out=sxy_sb[:, :], in_=ps[:, :])

                    pg = psG2.tile([C, P2_TILE], F32, tag="pg", name="pg")
                    nc.tensor.matmul(pg[:, :], s_wgate[:, :], sxy_sb[:, :], start=True, stop=True)
                    gat_sb = p2.tile([C, P2_TILE], BF16, tag="gat", name="gat_sb")
                    nc.scalar.activation(out=gat_sb[:, :], in_=pg[:, :], func=AF.Gelu,
                                         bias=s_bgate[:, :], scale=1.0)
                    gated_sb = p2.tile([C, P2_TILE], BF16, tag="gated", name="gated_sb")
                    nc.vector.tensor_mul(gated_sb[:, :], gat_sb[:, :], sxy_sb[:, :])

                    pdn = psD.tile([64, P2_TILE], F32, tag="pd2", name="pdn")
                    nc.tensor.matmul(pdn[:, :], s_wdown[:, :], gated_sb[:, :],
                                     start=True, stop=True)
                    d_sb = p2.tile([64, P2_TILE], BF16, tag="dsb", name="d_sb")
                    nc.vector.tensor_scalar_add(out=d_sb[:, :], in0=pdn[:, :],
                                                scalar1=s_bdown[:, :])

                    # out = (W_proj A P) v + (W_proj W_up) d  (+ W_proj b_up bias)
                    po = psO.tile([C, P2_TILE], F32, tag="po", name="po")
                    nc.tensor.matmul(po[:, :], wpa_bf[:, :], v_sl, start=True, stop=False)
                    nc.tensor.matmul(po[:, :], s_wpu[:, :], d_sb[:, :],
                                     start=False, stop=True)
                    outf = p2.tile([C, P2_TILE], F32, tag="outf", name="outf")
                    nc.scalar.activation(out=outf[:, :], in_=po[:, :], func=AF.Identity,
                                         bias=s_bpu[:, :], scale=1.0)
                    nc.sync.dma_start(out=out1_h[:, c0:c0 + P2_TILE], in_=outf[:, :])


_NC_CACHE = None


def _get_nc():
    global _NC_CACHE
    if _NC_CACHE is None:
        _NC_CACHE = build_bass()
    return _NC_CACHE


def _host_inputs(x, temperature, w_qkv, w_dw, w_proj, w_gate, b_gate,
                 w_down, b_down, w_up, b_up):
    f = np.float32
    x = np.asarray(x, f).reshape(B, C, L)
    w_qkv = np.asarray(w_qkv, f)
    w_dw = np.asarray(w_dw, f)
    temperature = np.asarray(temperature, f)

    shared = {}
    packbf = np.zeros((C, 12 * C + 64), np.float32)
    for ti, (dy, dx) in enumerate(PE_TAPS):
        wt = w_dw[2 * C:3 * C, 0, dy + 1, dx + 1]
        packbf[:, ti * C:(ti + 1) * C] = (w_qkv[2 * C:3 * C, :] * wt[:, None]).T
    packbf[:, 9 * C:10 * C] = np.asarray(w_gate, f).T
    packbf[:, 10 * C:11 * C] = np.asarray(w_proj, f).T
    packbf[:, 11 * C:12 * C] = np.eye(C, dtype=f)
    packbf[:, 12 * C:12 * C + 64] = np.asarray(w_down, f).T
    shared["packbf"] = packbf.astype(BD)
    wpu = np.asarray(w_proj, f) @ np.asarray(w_up, f)          # (C, 64)
    shared["wpu_t"] = np.ascontiguousarray(wpu.T).astype(BD)   # (64, C)

    pack8 = np.zeros((C, 20 * C), np.float32)
    for p in range(2):
        for ti, (dy, dx) in enumerate(PE_TAPS):
            wt = w_dw[p * C:(p + 1) * C, 0, dy + 1, dx + 1]
            pack8[:, (p * 10 + ti) * C:(p * 10 + ti + 1) * C] = \
                (w_qkv[p * C:(p + 1) * C, :] * wt[:, None]).T * W8SCALE
    shared["pack8"] = pack8.astype(F8)

    # exact channel means of dwconv(Wq x) via rectangle sums (linear in x),
    # and exact per-image row sums for q and k planes
    xr = x.reshape(B, C, H, W).astype(np.float64)
    wq = w_qkv[:C, :].astype(np.float64)
    wk = w_qkv[C:2 * C, :].astype(np.float64)
    wdw_q = w_dw[:C, 0].astype(np.float64)
    wdw_k = w_dw[C:2 * C, 0].astype(np.float64)
    mean = np.zeros(C, np.float64)
    rs_q = np.zeros((B, C), np.float64)
    rs_k = np.zeros((B, C), np.float64)
    for dy in (-1, 0, 1):
        for dx in (-1, 0, 1):
            y0, y1 = max(0, dy), min(H - 1, H - 1 + dy)
            x0, x1 = max(0, dx), min(W - 1, W - 1 + dx)
            rect_b = xr[:, :, y0:y1 + 1, x0:x1 + 1].sum(axis=(2, 3))   # (B, C)
            rect = rect_b.sum(axis=0)
            mean += wdw_q[:, dy + 1, dx + 1] * (wq @ rect)
            rs_q += wdw_q[None, :, dy + 1, dx + 1] * (rect_b @ wq.T)
            rs_k += wdw_k[None, :, dy + 1, dx + 1] * (rect_b @ wk.T)
    mean /= float(B * L)
    idx_order = np.argsort(-mean, kind="stable")
    rank = np.empty(C, np.int64)
    rank[idx_order] = np.arange(C)

    gid = np.zeros(C, np.int64)
    s = 0
    for gi, g in enumerate(GROUP_SIZES):
        gid[s:s + g] = gi
        s += g
    same = (gid[:, None] == gid[None, :])
    idx = np.arange(C)
    packf = np.zeros((C, 4 * C + 15), f)
    packf[:, 0:C] = same.astype(f)
    packf[:, C:2 * C] = np.where(same, 0.0, -30000.0)
    packf[:, 2 * C:3 * C] = np.tile(idx[None, :], (C, 1))
    packf[:, 3 * C:4 * C] = np.tile(rank[None, :], (C, 1)).astype(f)
    packf[:, 4 * C + 0] = np.asarray(b_gate, f)
    packf[:, 4 * C + 1] = np.asarray(b_up, f)
    packf[:, 4 * C + 2] = np.asarray(w_proj, f) @ np.asarray(b_up, f)
    packf[:, 4 * C + 3] = temperature[gid, 0, 0]
    packf[:, 4 * C + 4] = rank.astype(f)
    packf[:, 4 * C + 5] = idx.astype(f)
    shared["packf"] = packf
    shared["bdown"] = np.asarray(b_down, f).reshape(64, 1)

    xg = np.zeros((B, C, H + 2, GAPW), np.float32)
    xg[:, :, 1:H + 1, 2:] = x.reshape(B, C, H, W)
    xg = xg.reshape(B, C, (H + 2) * GAPW)
    in_maps = []
    for i in range(NCORES):
        m = dict(shared)
        m["xbf"] = xg[i].astype(BD)
        m["x8"] = xg[i].astype(F8)
        m["rs"] = np.stack([rs_q[i], rs_k[i]], axis=1).astype(f)
        in_maps.append(m)
    return in_maps


def _assemble(results):
    out = np.zeros((B, C, H, W), np.float32)
    cache = np.zeros((B, C, H, W), np.float32)
    for i in range(NCORES):
        out[i] = np.asarray(results[i]["out1"], np.float32).reshape(C, H, W)
        st = np.asarray(results[i]["stats"], np.float32)
        v0 = st[:, 0]
        rank = np.rint(st[:, 1]).astype(np.int64)
        idx_dev = np.argsort(rank)
        mt = v0[idx_dev]
        s = 0
        gms = []
        for g in GROUP_SIZES:
            gm = mt[s:s + g]
            s += g
            rep = max(1, C // g)
            gm = np.tile(gm, rep)
            if gm.shape[0] >= C:
                gm = gm[:C]
            else:
                gm = np.pad(gm, (0, C - gm.shape[0]))
            gms.append(gm)
        acc = np.mean(np.stack(gms, 0), 0)
        cache[i] = np.broadcast_to((acc * 0.9)[:, None, None], (C, H, W))
    return out, cache


def kernel(**inputs):
    nc = _get_nc()
    in_maps = _host_inputs(**inputs)
    res = run_bass_kernel_spmd(nc, in_maps, list(range(NCORES)))
    return _assemble(res.results)


if __name__ == "__main__":
    rng = np.random.default_rng(0)
    dummy = {
        "x": rng.standard_normal((B, C, H, W), dtype=np.float32),
        "temperature": np.ones((4, 1, 1), np.float32),
        "w_qkv": rng.standard_normal((3 * C, C), dtype=np.float32) * 0.02,
        "w_dw": rng.standard_normal((3 * C, 1, 3, 3), dtype=np.float32) * 0.02,
        "w_proj": rng.standard_normal((C, C), dtype=np.float32) * 0.02,
        "w_gate": rng.standard_normal((C, C), dtype=np.float32) * 0.02,
        "b_gate": np.zeros(C, np.float32),
        "b_down": np.zeros(C // 2, np.float32),
        "w_down": rng.standard_normal((C // 2, C), dtype=np.float32) * 0.02,
        "w_up": rng.standard_normal((C, C // 2), dtype=np.float32) * 0.02,
        "b_up": np.zeros(C, np.float32),
    }
    o, c = kernel(**dummy)
    print("out", o.shape, o.dtype, "cache", c.shape, c.dtype)
